# revision 19
# baseline (speedup 1.0000x reference)
"""Trainium2 Bass kernel for nn_CCA2_18786186953483 (dense_transformer).

Reference computation (per batch element b):
  q   = (x @ Wq + bq) * hd^-0.5, split into 8 heads of 64
  x_1 = cross_attn(q, fore_x, Wkv1, bkv1)   # S=2048
  x_2 = cross_attn(q, post_x, Wkv2, bkv2)   # S=512
  out = concat([x, x_1, x_2], -1) @ Wp + bp

Sharding: data-parallel over batch B=8 across the 8 NeuronCores (one batch
element per core); weights replicated.

Kernel strategy (per core):
  * fp8e4m3 DoubleRow matmuls (0.5 cycles/row) for the q / kv projections and
    the x1/x2 parts of the output projection; bf16 for logits, AV, and the
    x part of the output projection.
  * k-bias dropped entirely (adds a per-(head,l) constant to logits, which
    cancels in softmax); v-bias and bp folded into a single bias row added
    via a K=1 ones-row matmul at projection time.
  * Attention AV is computed output-transposed ([l, d] layout) with V
    augmented by a ones column, so softmax denominators land as a per-
    partition column; normalization is a per-partition tensor_scalar
    multiply by the reciprocal, fused into the PSUM->SBUF evacuation.
  * The normalized [l, d] heads are transposed back to [c, l] on the PE
    (identity matmul) for the output projection.
  * Softmax exp is split across two engines: the Activation engine's native
    Exp, and a Schraudolph fast-exp on the Vector engine (fused
    multiply-add to int16, bit-reinterpreted as bf16; ~1.7% rms error).
  * GPSIMD (Pool) does all SBUF-side fp8/bf16 staging casts.
"""

import numpy as np

B, L, C = 8, 1024, 512
H, HD = 8, 64
CIN = 256
S1, S2 = 2048, 512
C3 = 3 * C
P = 128
N_CORES = 8

# Schraudolph fast-exp constants for exp(x) with x the (already 0.125-scaled)
# logits: E = bitcast_bf16(int16(round(x * C1 + C2)))
SCH_C1 = 184.66496280558492  # 128 * log2(e)
SCH_C2 = 16250.5

_CACHE = {}

# exp engine assignment: every EXP_DVE_EVERY-th slab goes to DVE (Schraudolph).
# Set to 0 to disable DVE exp entirely.
EXP_DVE_EVERY = 3
PIPELINED_AV = True


def _split_multi_waits(nc, mybir):
    """This container's walrus build supports only ONE sync-wait command per
    instruction ("Too many sync wait commands").  Tile emits instructions
    with several waits; split the extras onto same-engine NOPs placed
    immediately before the instruction (same engine stream => identical
    blocking semantics)."""
    ctr = 0
    for f in nc.m.functions:
        for blk in f.blocks:
            insts = blk.instructions
            if not any(
                i.sync_info is not None
                and i.sync_info.on_wait
                and len(i.sync_info.on_wait) > 1
                for i in insts
            ):
                continue
            new_list = []
            for inst in insts:
                si = inst.sync_info
                waits = list(si.on_wait) if (si is not None and si.on_wait) else []
                if len(waits) > 1:
                    # A wait on the instruction's OWN engine semaphore is
                    # always already satisfied (engines execute and complete
                    # strictly in order), so it is pure dispatch overhead.
                    # Drop it only when doing so avoids emitting split NOPs.
                    own = str(inst.engine).split(".")[-1] + "_"
                    kept = [
                        w
                        for w in waits
                        if not (w.ant_name or "").startswith(own)
                    ]
                    if kept:
                        waits = kept
                if len(waits) == 1:
                    inst.sync_info = mybir.SyncInfo(
                        on_wait=waits, on_update=list(si.on_update or [])
                    )
                if len(waits) > 1:
                    for w in waits[:-1]:
                        ctr += 1
                        new_list.append(
                            mybir.InstNoOp(
                                name=f"I-waitsplit-{ctr}",
                                engine=inst.engine,
                                bass_nofuse=True,
                                sync_info=mybir.SyncInfo(on_wait=[w], on_update=[]),
                            )
                        )
                    inst.sync_info = mybir.SyncInfo(
                        on_wait=[waits[-1]], on_update=list(si.on_update or [])
                    )
                new_list.append(inst)
            insts[:] = new_list
    return ctr


def _build(split=True):
    import contextlib

    import concourse.bass as bass
    import concourse.tile as tile
    from concourse import mybir
    from concourse.masks import make_identity

    f32 = mybir.dt.float32
    bf16 = mybir.dt.bfloat16
    fp8 = mybir.dt.float8e4
    i16 = mybir.dt.int16
    EXPF = mybir.ActivationFunctionType.Exp
    IDF = mybir.ActivationFunctionType.Identity
    CPF = mybir.ActivationFunctionType.Copy
    DR = mybir.MatmulPerfMode.DoubleRow
    MUL = mybir.AluOpType.mult
    ADD = mybir.AluOpType.add

    nc = bass.Bass("TRN2")

    xd = nc.dram_tensor("x", [L, C], f32, kind="ExternalInput")
    fored = nc.dram_tensor("fore_x", [S1, CIN], f32, kind="ExternalInput")
    postd = nc.dram_tensor("post_x", [S2, CIN], f32, kind="ExternalInput")
    Wqd = nc.dram_tensor("Wq", [C, C], f32, kind="ExternalInput")
    bqd = nc.dram_tensor("bq", [C], f32, kind="ExternalInput")
    Wkv1d = nc.dram_tensor("Wkv1", [CIN, 2 * C], f32, kind="ExternalInput")
    bkv1d = nc.dram_tensor("bkv1", [2 * C], f32, kind="ExternalInput")
    Wkv2d = nc.dram_tensor("Wkv2", [CIN, 2 * C], f32, kind="ExternalInput")
    bkv2d = nc.dram_tensor("bkv2", [2 * C], f32, kind="ExternalInput")
    Wpd = nc.dram_tensor("Wp", [C3, C], f32, kind="ExternalInput")
    bpd = nc.dram_tensor("bp", [C], f32, kind="ExternalInput")
    outd = nc.dram_tensor("out", [L, C], f32, kind="ExternalOutput")

    with tile.TileContext(nc) as tc, contextlib.ExitStack() as ctx:
        stage = ctx.enter_context(tc.tile_pool(name="stage", bufs=3))
        tbfp = ctx.enter_context(tc.tile_pool(name="tbfp", bufs=1))
        epool = ctx.enter_context(tc.tile_pool(name="epool", bufs=11))
        work = ctx.enter_context(tc.tile_pool(name="work", bufs=3))
        persist = ctx.enter_context(tc.tile_pool(name="persist", bufs=1))
        slab = ctx.enter_context(tc.tile_pool(name="slab", bufs=2, space="PSUM"))
        avp = ctx.enter_context(tc.tile_pool(name="avp", bufs=2, space="PSUM"))
        aux = ctx.enter_context(tc.tile_pool(name="aux", bufs=2, space="PSUM"))
        dram = ctx.enter_context(tc.tile_pool(name="dram", bufs=1, space="DRAM"))

        # round-robin assignment of evac-ish work across ACT / DVE
        _exp_ctr = [0]

        def exp_engine():
            _exp_ctr[0] += 1
            if not EXP_DVE_EVERY:
                return "act"
            return "dve" if _exp_ctr[0] % EXP_DVE_EVERY == 0 else "act"

        def emit_exp(e_tile, slab_view, eng):
            if eng == "act":
                nc.scalar.activation(e_tile, slab_view, EXPF)
            else:
                nc.vector.tensor_scalar(
                    e_tile.bitcast(i16), slab_view, SCH_C1, SCH_C2, MUL, ADD
                )

        # ---------------- phase 0: x transpose + q projection ------------
        ident = persist.tile([P, P], bf16, tag="ident")
        make_identity(nc, ident[:])
        warm = aux.tile([P, 512], f32, tag="aux", name="warm")
        for _ in range(30):
            nc.tensor.matmul(warm[:, :P], ident[:], ident[:], start=True, stop=True)

        xT = [persist.tile([P, L], bf16, tag=f"xT{ci}", name=f"xT{ci}") for ci in range(4)]
        xv = xd.rearrange("(n p) c -> p n c", p=P)

        def x_quarter(qi):
            st = stage.tile([P, 2, C], f32, tag="stage_x")
            nc.sync.dma_start(st[:], xv[:, 2 * qi : 2 * qi + 2])
            cst = stage.tile([P, 2, C], bf16, tag="stage_xbf")
            nc.scalar.copy(cst[:], st[:])
            for ci in range(4):
                ps = aux.tile([P, 512], bf16, tag="aux", name="xtp")
                for n in range(2):
                    nc.tensor.transpose(
                        ps[:, P * n : P * (n + 1)],
                        cst[:, n, P * ci : P * (ci + 1)],
                        ident[:],
                    )
                nc.vector.tensor_copy(xT[ci][:, 256 * qi : 256 * (qi + 1)], ps[:, :256])

        for qi in range(4):
            x_quarter(qi)

        # xT8: fp8 copy of xT in DoubleRow ktile layout [P, 4, L]
        xT8 = persist.tile([P, 4, L], fp8, tag="xT8")
        for ci in range(4):
            nc.gpsimd.tensor_copy(xT8[:, ci, :], xT[ci][:])

        # Wq -> fp8 [P, 4, C];  bq column tile [P, 4] (prescaled by 0.125)
        wq8 = persist.tile([P, 4, C], fp8, tag="wq8")
        wqv = Wqd.rearrange("(n p) c -> p n c", p=P)
        for i in range(2):
            st = stage.tile([P, 2, C], f32, tag="stage_w")
            nc.sync.dma_start(st[:], wqv[:, 2 * i : 2 * i + 2])
            nc.gpsimd.tensor_copy(wq8[:, 2 * i : 2 * i + 2, :], st[:])
        bq_sb = persist.tile([P, 4], f32, tag="bq")
        nc.sync.dma_start(bq_sb[:], bqd.rearrange("(o p) -> p o", p=P))
        bq_s = persist.tile([P, 4], f32, tag="bqs")
        nc.vector.tensor_scalar(bq_s[:], bq_sb[:], 0.125, None, MUL)

        # q projection (fp8 DR): qT[cq, l] = sum_c Wq[c, cq] xT[c, l]
        # evacuated with scale 0.125 and bias bq*0.125 -> bf16
        qT = [persist.tile([P, L], bf16, tag=f"qT{i}", name=f"qT{i}") for i in range(4)]
        for cq in range(4):
            ps = slab.tile([P, 2, 512], f32, tag="slab", name="qps")
            psv = ps.rearrange("p a n -> p (a n)")
            for lq in range(4):
                for j in range(2):
                    nc.tensor.matmul(
                        psv[:, 256 * lq : 256 * (lq + 1)],
                        wq8[:, 2 * j : 2 * j + 2, P * cq : P * (cq + 1)],
                        xT8[:, 2 * j : 2 * j + 2, 256 * lq : 256 * (lq + 1)],
                        start=(j == 0),
                        stop=(j == 1),
                        perf_mode=DR,
                    )
            nc.scalar.activation(
                qT[cq][:], psv[:], IDF, scale=0.125, bias=bq_s[:, cq : cq + 1]
            )

        # ---------------- phase 0b: post_x / fore_x transposed loads ------
        def cast_to_dram(src, rows, cols):
            # casts on Pool (SBUF->SBUF) so ACT/DVE stay free for attention
            n = rows // P
            bf_dram = dram.tile([rows, cols], bf16)
            src_v = src.rearrange("(n p) c -> p n c", p=P)
            dst_v = bf_dram.rearrange("(n p) c -> p n c", p=P)
            step = 4
            for i in range(0, n, step):
                m = min(step, n - i)
                st = stage.tile([P, step, cols], f32, tag="stage_in")
                nc.sync.dma_start(st[:, :m], src_v[:, i : i + m])
                cst = stage.tile([P, step, cols], bf16, tag="stage_bf")
                nc.gpsimd.tensor_copy(cst[:, :m], st[:, :m])
                nc.sync.dma_start(dst_v[:, i : i + m], cst[:, :m])
            return bf_dram

        def transpose_load_fp8(bf_dram, rows, tagname):
            # -> [P, 2, rows] fp8 (ktile layout for DoubleRow)
            t_bf = tbfp.tile([P, 2, rows], bf16, tag="t_bf", name="t_bf")
            for ci in range(2):
                nc.sync.dma_start_transpose(
                    t_bf[:, ci, :], bf_dram[:, P * ci : P * (ci + 1)]
                )
            t8 = persist.tile([P, 2, rows], fp8, tag=tagname, name=tagname)
            nc.gpsimd.tensor_copy(t8[:, 0, :], t_bf[:, 0, :])
            nc.gpsimd.tensor_copy(t8[:, 1, :], t_bf[:, 1, :])
            return t8

        post_bf = cast_to_dram(postd, S2, CIN)
        postT8 = transpose_load_fp8(post_bf, S2, "postT8")

        def load_wkv8(src, tagname):
            w8 = persist.tile([P, 2, 2 * C], fp8, tag=tagname, name=tagname)
            srcv = src.rearrange("(n p) c -> p n c", p=P)
            for ci in range(2):
                st = stage.tile([P, 2 * C], f32, tag="stage_wkv", name="st")
                nc.sync.dma_start(st[:], srcv[:, ci])
                nc.gpsimd.tensor_copy(w8[:, ci, :], st[:])
            return w8

        wkv28 = load_wkv8(Wkv2d, "wkv28")

        # ---------------- kv producers ------------------------------------
        def make_kT(srcT8, w8, S, kname, evac_eng):
            # kT[cq, s] = sum_c Wkv[c, cq] srcT[c, s]   (no bias: cancels)
            tiles = []
            for cq in range(4):
                t = persist.tile([P, S], bf16, tag=f"{kname}{cq}", name=f"{kname}{cq}")
                for off in range(0, S, 1024):
                    w = min(1024, S - off)
                    ps = slab.tile([P, 2, 512], f32, tag="slab", name="kps")
                    psv = ps.rearrange("p a n -> p (a n)")
                    for sb in range(w // 256):
                        nc.tensor.matmul(
                            psv[:, 256 * sb : 256 * (sb + 1)],
                            w8[:, :, P * cq : P * (cq + 1)],
                            srcT8[:, :, off + 256 * sb : off + 256 * (sb + 1)],
                            start=True,
                            stop=True,
                            perf_mode=DR,
                        )
                    dst = t[:, off : off + w]
                    if evac_eng == "alt":
                        evac_eng_i = "act" if (cq + off // 1024) % 2 == 0 else "dve"
                    else:
                        evac_eng_i = evac_eng
                    if evac_eng_i == "act":
                        nc.scalar.copy(dst, psv[:, :w])
                    else:
                        nc.vector.tensor_copy(dst, psv[:, :w])
                tiles.append(t)
            return tiles

        def make_v(srcT8, w8, S, vname):
            # V_aug tiles per si-pair: [P, 2, H, HD+1] bf16 with ones col
            tiles = []
            for u in range(S // 256):
                vt = persist.tile([P, 2, H, HD + 1], bf16, tag=f"{vname}{u}", name=f"{vname}{u}")
                nc.gpsimd.memset(vt[:, :, :, HD : HD + 1], 1.0)
                tiles.append(vt)
            for si in range(S // P):
                ps = aux.tile([P, 512], f32, tag="aux", name="vps")
                for dh in range(2):
                    nc.tensor.matmul(
                        ps[:, 256 * dh : 256 * (dh + 1)],
                        srcT8[:, :, P * si : P * (si + 1)],
                        w8[:, :, C + 256 * dh : C + 256 * (dh + 1)],
                        start=True,
                        stop=True,
                        perf_mode=DR,
                    )
                psv = ps.rearrange("p (h d) -> p h d", h=H)
                dst = tiles[si // 2][:, si % 2, :, 0:HD]
                if si % 2 == 0:
                    nc.vector.tensor_copy(dst, psv[:])
                else:
                    nc.scalar.copy(dst, psv[:])
            return tiles

        kT2 = make_kT(postT8, wkv28, S2, "k2T", "act")
        v2 = make_v(postT8, wkv28, S2, "v2")

        # start the fore_x DRAM round-trip now (DMA + Pool only); the
        # dependent matmuls are emitted after attention(S2) so the PE
        # doesn't stall on these DMAs.
        fore_bf = cast_to_dram(fored, S1, CIN)
        wkv18 = load_wkv8(Wkv1d, "wkv18")

        # ---------------- attention ---------------------------------------
        x1T8 = [persist.tile([P, 2, L], fp8, tag=f"x1T8_{t}", name=f"x1T8_{t}") for t in range(2)]
        x2T8 = [persist.tile([P, 2, L], fp8, tag=f"x2T8_{t}", name=f"x2T8_{t}") for t in range(2)]

        def attention(S, kT, v_sb, xT8_out, after_unit=None):
            # software-pipelined: per si-pair, logits+exp for both heads of
            # the pair are emitted, then the PREVIOUS si-pair's AV matmuls.
            # E tiles are consumed one pipeline stage later, so only ~6 are
            # live and the PE never has to wait for a whole head's exps.
            nsp = S // 256  # number of si-pairs
            for p in range(4):  # head pairs
                for lh in range(2):
                    av_sb = work.tile([P, 4, 2, HD], bf16, tag="av_sb")
                    # PSUM zero-region semantics: a start marks the whole
                    # 2KB window of the tile as pending-zero, so emit start
                    # ONLY on the tile's very first matmul and stop only on
                    # its last; later regions' first writes are handled by
                    # the pending-zero overwrite.
                    avs = [
                        avp.tile([P, 4, HD + 1], f32, tag="av", name=f"av{hh}")
                        for hh in range(2)
                    ]
                    kt = kT[p]
                    qt = qT[p]
                    e_cur = [None, None]

                    def emit_logits_exp(sp):
                        for hh in range(2):
                            po = 64 * hh
                            ps = slab.tile([P, 2, 512], f32, tag="slab", name="lg")
                            for j in range(2):
                                si = 2 * sp + j
                                nc.tensor.matmul(
                                    ps[:, j, :],
                                    kt[po : po + HD, P * si : P * (si + 1)],
                                    qt[po : po + HD, 512 * lh : 512 * (lh + 1)],
                                    start=True,
                                    stop=True,
                                )
                            et = epool.tile([P, 2, 512], bf16, tag="e")
                            emit_exp(
                                et[:], ps.rearrange("p a n -> p (a n)"), exp_engine()
                            )
                            e_cur[hh] = et

                    def emit_av(sp, e_pair):
                        for hh in range(2):
                            for lb in range(4):
                                for j in range(2):
                                    nc.tensor.matmul(
                                        avs[hh][:, lb, :],
                                        e_pair[hh][:, j, P * lb : P * (lb + 1)],
                                        v_sb[sp][:, j, 2 * p + hh, :],
                                        start=(sp == 0 and lb == 0 and j == 0),
                                        stop=(sp == nsp - 1 and lb == 3 and j == 1),
                                        skip_group_check=True,
                                    )

                    if globals()['PIPELINED_AV']:
                        prev = None
                        for sp in range(nsp):
                            emit_logits_exp(sp)
                            if prev is not None:
                                emit_av(prev[0], prev[1])
                            prev = (sp, list(e_cur))
                        emit_av(prev[0], prev[1])
                    else:
                        all_e = []
                        for sp in range(nsp):
                            emit_logits_exp(sp)
                            all_e.append(list(e_cur))
                        for hh in range(2):
                            for lb in range(4):
                                for sp in range(nsp):
                                    for j in range(2):
                                        nc.tensor.matmul(
                                            avs[hh][:, lb, :],
                                            all_e[sp][hh][:, j, P * lb : P * (lb + 1)],
                                            v_sb[sp][:, j, 2 * p + hh, :],
                                            start=(lb == 0 and sp == 0 and j == 0),
                                            stop=(lb == 3 and sp == nsp - 1 and j == 1),
                                            skip_group_check=True,
                                        )

                    # normalize: per-partition reciprocal of ones column
                    for hh in range(2):
                        dnr = work.tile([P, 4], f32, tag="dnr")
                        nc.vector.reciprocal(
                            dnr[:],
                            avs[hh][:, :, HD : HD + 1].rearrange("p a o -> p (a o)"),
                        )
                        for lb in range(4):
                            dst = av_sb[:, lb, hh, :]
                            if (lb + 2 * hh) % 2 == 0:
                                nc.scalar.activation(
                                    dst, avs[hh][:, lb, 0:HD], CPF,
                                    scale=dnr[:, lb : lb + 1],
                                )
                            else:
                                nc.vector.tensor_scalar(
                                    dst, avs[hh][:, lb, 0:HD], dnr[:, lb : lb + 1],
                                    None, MUL,
                                )
                    # transpose back to [c, l] and store as fp8
                    txp = aux.tile([P, 512], bf16, tag="aux", name="txp")
                    for lb in range(4):
                        nc.tensor.transpose(
                            txp[:, P * lb : P * (lb + 1)],
                            av_sb[:, lb, :, :].rearrange("p a d -> p (a d)"),
                            ident[:],
                        )
                    dst = xT8_out[p // 2][:, p % 2, 512 * lh : 512 * (lh + 1)]
                    if p % 2 == 0:
                        nc.scalar.copy(dst, txp[:])
                    else:
                        nc.vector.tensor_copy(dst, txp[:])
                    if after_unit is not None:
                        after_unit(2 * p + lh)

        attention(S2, kT2, v2, x2T8)

        foreT8 = transpose_load_fp8(fore_bf, S1, "foreT8")
        kT1 = make_kT(foreT8, wkv18, S1, "k1T", "alt")
        v1 = make_v(foreT8, wkv18, S1, "v1")

        # ---------------- Wp + bias row -----------------------------------
        wpv = Wpd.rearrange("(n p) c -> p n c", p=P)
        wpx = persist.tile([P, 4, C], bf16, tag="wpx")  # x part, bf16
        for i in range(2):
            st = stage.tile([P, 2, C], f32, tag="stage_w")
            nc.sync.dma_start(st[:], wpv[:, 2 * i : 2 * i + 2])
            nc.gpsimd.tensor_copy(wpx[:, 2 * i : 2 * i + 2, :], st[:])
        wp18 = persist.tile([P, 2, 2, C], fp8, tag="wp18")  # x1 part
        wp28 = persist.tile([P, 2, 2, C], fp8, tag="wp28")  # x2 part
        for t8, base in ((wp18, 4), (wp28, 8)):
            for i in range(2):
                st = stage.tile([P, 2, C], f32, tag="stage_w")
                nc.sync.dma_start(st[:], wpv[:, base + 2 * i : base + 2 * (i + 1)])
                nc.gpsimd.tensor_copy(t8[:, i, :, :], st[:])

        # bias row bpp = bp + bv1 @ Wp[C:2C] + bv2 @ Wp[2C:3C]
        # bv scaled by 16 before fp8 cast (values ~0.02 are subnormal in fp8)
        bv8 = persist.tile([P, 2, 4], fp8, tag="bv8")
        for bi, bd in ((0, bkv1d), (1, bkv2d)):
            bcol = stage.tile([P, 4], f32, tag="stage_bv")
            nc.sync.dma_start(bcol[:], bd[C : 2 * C].rearrange("(o p) -> p o", p=P))
            bscaled = stage.tile([P, 4], f32, tag="stage_bvs")
            nc.vector.tensor_scalar(bscaled[:], bcol[:], 16.0, None, MUL)
            nc.gpsimd.tensor_copy(bv8[:, bi, :], bscaled[:])
        bp_row = persist.tile([1, C], f32, tag="bp_row")
        nc.sync.dma_start(bp_row[:], bpd.rearrange("(o c) -> o c", o=1))
        bias_ps = aux.tile([P, 512], f32, tag="aux", name="biasps")
        for bi, w8 in ((0, wp18), (1, wp28)):
            for k in range(4):
                nc.tensor.matmul(
                    bias_ps[0:1, :],
                    bv8[:, bi, k : k + 1],
                    w8[:, k // 2, k % 2, :],
                    start=(bi == 0 and k == 0),
                    stop=(bi == 1 and k == 3),
                )
        bpp = persist.tile([1, C], bf16, tag="bpp")
        nc.vector.scalar_tensor_tensor(
            bpp[:], bias_ps[0:1, :], 1.0 / 16.0, bp_row[:], MUL, ADD
        )
        onescol = persist.tile([1, P], bf16, tag="onescol")
        nc.vector.memset(onescol[:], 1.0)


        # ---------------- output projection -------------------------------
        # x + x2 + bias partials are emitted at attention-1 unit boundaries
        # (they only need x2T8/xT/bpp); the x1 part runs in the tail.
        acc_sb = [
            persist.tile([P, C], f32, tag=f"acc{li}", name=f"acc{li}")
            for li in range(8)
        ]

        def proj_partial(li):
            ps = aux.tile([P, 512], f32, tag="aux", name="prps")
            nc.tensor.matmul(
                ps[0:P, :], onescol[:, :], bpp[:, :], start=True, stop=False
            )
            for ki in range(4):
                nc.tensor.matmul(
                    ps[:],
                    xT[ki][:, P * li : P * (li + 1)],
                    wpx[:, ki, :],
                    start=False,
                    stop=False,
                )
            for t in range(2):
                for nh in range(2):
                    nc.tensor.matmul(
                        ps[:, 256 * nh : 256 * (nh + 1)],
                        x2T8[t][:, :, P * li : P * (li + 1)],
                        wp28[:, t, :, 256 * nh : 256 * (nh + 1)],
                        start=False,
                        stop=(t == 1 and nh == 1),
                        perf_mode=DR,
                    )
            if li % 2 == 0:
                nc.scalar.copy(acc_sb[li][:], ps[:])
            else:
                nc.vector.tensor_copy(acc_sb[li][:], ps[:])

        attention(S1, kT1, v1, x1T8, after_unit=proj_partial)

        for li in range(8):
            ps = aux.tile([P, 512], f32, tag="aux", name="prps2")
            for t in range(2):
                for nh in range(2):
                    nc.tensor.matmul(
                        ps[:, 256 * nh : 256 * (nh + 1)],
                        x1T8[t][:, :, P * li : P * (li + 1)],
                        wp18[:, t, :, 256 * nh : 256 * (nh + 1)],
                        start=(t == 0 and nh == 0),
                        stop=(t == 1 and nh == 1),
                        skip_group_check=True,
                        perf_mode=DR,
                    )
            ot = work.tile([P, C], f32, tag="ot")
            if li % 2 == 0:
                nc.scalar.activation(ot[:], ps[:], IDF, bias=0.0)
                nc.vector.tensor_tensor(ot[:], ot[:], acc_sb[li][:], ADD)
            else:
                nc.vector.tensor_tensor(ot[:], ps[:], acc_sb[li][:], ADD)
            nc.sync.dma_start(outd[P * li : P * (li + 1), :], ot[:])

    if split:
        _split_multi_waits(nc, mybir)
    return nc


def _get_nc():
    if "nc" not in _CACHE:
        _CACHE["nc"] = _build()
    return _CACHE["nc"]


def kernel(**inputs):
    from concourse.bass_utils import run_bass_kernel_spmd

    nc = _get_nc()
    shared = {
        k: np.ascontiguousarray(inputs[k], dtype=np.float32)
        for k in ("Wq", "bq", "Wkv1", "bkv1", "Wkv2", "bkv2", "Wp", "bp")
    }
    in_maps = []
    for b in range(N_CORES):
        m = dict(shared)
        m["x"] = np.ascontiguousarray(inputs["x"][b], dtype=np.float32)
        m["fore_x"] = np.ascontiguousarray(inputs["fore_x"][b], dtype=np.float32)
        m["post_x"] = np.ascontiguousarray(inputs["post_x"][b], dtype=np.float32)
        in_maps.append(m)
    res = run_bass_kernel_spmd(nc, in_maps, core_ids=list(range(N_CORES)))
    out = np.stack([res.results[b]["out"] for b in range(N_CORES)], axis=0)
    return out.astype(np.float32)


# revision 20
# speedup vs baseline: 1.0199x; 1.0199x over previous
"""Trainium2 Bass kernel for nn_CCA2_18786186953483 (dense_transformer).

Reference computation (per batch element b):
  q   = (x @ Wq + bq) * hd^-0.5, split into 8 heads of 64
  x_1 = cross_attn(q, fore_x, Wkv1, bkv1)   # S=2048
  x_2 = cross_attn(q, post_x, Wkv2, bkv2)   # S=512
  out = concat([x, x_1, x_2], -1) @ Wp + bp

Sharding: data-parallel over batch B=8 across the 8 NeuronCores (one batch
element per core); weights replicated.

Kernel strategy (per core):
  * fp8e4m3 DoubleRow matmuls (0.5 cycles/row) for the q / kv projections and
    the x1/x2 parts of the output projection; bf16 for logits, AV, and the
    x part of the output projection.
  * k-bias dropped entirely (adds a per-(head,l) constant to logits, which
    cancels in softmax); v-bias and bp folded into a single bias row added
    via a K=1 ones-row matmul at projection time.
  * Attention AV is computed output-transposed ([l, d] layout) with V
    augmented by a ones column, so softmax denominators land as a per-
    partition column; normalization is a per-partition tensor_scalar
    multiply by the reciprocal, fused into the PSUM->SBUF evacuation.
  * The normalized [l, d] heads are transposed back to [c, l] on the PE
    (identity matmul) for the output projection.
  * Softmax exp is split across two engines: the Activation engine's native
    Exp, and a Schraudolph fast-exp on the Vector engine (fused
    multiply-add to int16, bit-reinterpreted as bf16; ~1.7% rms error).
  * GPSIMD (Pool) does all SBUF-side fp8/bf16 staging casts.
"""

import numpy as np

B, L, C = 8, 1024, 512
H, HD = 8, 64
CIN = 256
S1, S2 = 2048, 512
C3 = 3 * C
P = 128
N_CORES = 8

# Schraudolph fast-exp constants for exp(x) with x the (already 0.125-scaled)
# logits: E = bitcast_bf16(int16(round(x * C1 + C2)))
SCH_C1 = 184.66496280558492  # 128 * log2(e)
SCH_C2 = 16250.5

_CACHE = {}

# exp engine assignment: DVE (Schraudolph) gets EXP_DVE_NUM of every
# EXP_DVE_DEN slabs; the rest go to ACT's native Exp.
EXP_DVE_NUM = 9
EXP_DVE_DEN = 20
PIPELINED_AV = True


def _split_multi_waits(nc, mybir):
    """This container's walrus build supports only ONE sync-wait command per
    instruction ("Too many sync wait commands").  Tile emits instructions
    with several waits; split the extras onto same-engine NOPs placed
    immediately before the instruction (same engine stream => identical
    blocking semantics)."""
    ctr = 0
    for f in nc.m.functions:
        for blk in f.blocks:
            insts = blk.instructions
            if not any(
                i.sync_info is not None
                and i.sync_info.on_wait
                and len(i.sync_info.on_wait) > 1
                for i in insts
            ):
                continue
            new_list = []
            for inst in insts:
                si = inst.sync_info
                waits = list(si.on_wait) if (si is not None and si.on_wait) else []
                if len(waits) > 1:
                    # A wait on the instruction's OWN engine semaphore is
                    # always already satisfied (engines execute and complete
                    # strictly in order), so it is pure dispatch overhead.
                    # Drop it only when doing so avoids emitting split NOPs.
                    own = str(inst.engine).split(".")[-1] + "_"
                    kept = [
                        w
                        for w in waits
                        if not (w.ant_name or "").startswith(own)
                    ]
                    if kept:
                        waits = kept
                if len(waits) == 1:
                    inst.sync_info = mybir.SyncInfo(
                        on_wait=waits, on_update=list(si.on_update or [])
                    )
                if len(waits) > 1:
                    for w in waits[:-1]:
                        ctr += 1
                        new_list.append(
                            mybir.InstNoOp(
                                name=f"I-waitsplit-{ctr}",
                                engine=inst.engine,
                                bass_nofuse=True,
                                sync_info=mybir.SyncInfo(on_wait=[w], on_update=[]),
                            )
                        )
                    inst.sync_info = mybir.SyncInfo(
                        on_wait=[waits[-1]], on_update=list(si.on_update or [])
                    )
                new_list.append(inst)
            insts[:] = new_list
    return ctr


def _build(split=True):
    import contextlib

    import concourse.bass as bass
    import concourse.tile as tile
    from concourse import mybir
    from concourse.masks import make_identity

    f32 = mybir.dt.float32
    bf16 = mybir.dt.bfloat16
    fp8 = mybir.dt.float8e4
    i16 = mybir.dt.int16
    EXPF = mybir.ActivationFunctionType.Exp
    IDF = mybir.ActivationFunctionType.Identity
    CPF = mybir.ActivationFunctionType.Copy
    DR = mybir.MatmulPerfMode.DoubleRow
    MUL = mybir.AluOpType.mult
    ADD = mybir.AluOpType.add

    nc = bass.Bass("TRN2")

    xd = nc.dram_tensor("x", [L, C], f32, kind="ExternalInput")
    fored = nc.dram_tensor("fore_x", [S1, CIN], f32, kind="ExternalInput")
    postd = nc.dram_tensor("post_x", [S2, CIN], f32, kind="ExternalInput")
    Wqd = nc.dram_tensor("Wq", [C, C], f32, kind="ExternalInput")
    bqd = nc.dram_tensor("bq", [C], f32, kind="ExternalInput")
    Wkv1d = nc.dram_tensor("Wkv1", [CIN, 2 * C], f32, kind="ExternalInput")
    bkv1d = nc.dram_tensor("bkv1", [2 * C], f32, kind="ExternalInput")
    Wkv2d = nc.dram_tensor("Wkv2", [CIN, 2 * C], f32, kind="ExternalInput")
    bkv2d = nc.dram_tensor("bkv2", [2 * C], f32, kind="ExternalInput")
    Wpd = nc.dram_tensor("Wp", [C3, C], f32, kind="ExternalInput")
    bpd = nc.dram_tensor("bp", [C], f32, kind="ExternalInput")
    outd = nc.dram_tensor("out", [L, C], f32, kind="ExternalOutput")

    with tile.TileContext(nc) as tc, contextlib.ExitStack() as ctx:
        stage = ctx.enter_context(tc.tile_pool(name="stage", bufs=3))
        tbfp = ctx.enter_context(tc.tile_pool(name="tbfp", bufs=1))
        epool = ctx.enter_context(tc.tile_pool(name="epool", bufs=11))
        work = ctx.enter_context(tc.tile_pool(name="work", bufs=3))
        persist = ctx.enter_context(tc.tile_pool(name="persist", bufs=1))
        slab = ctx.enter_context(tc.tile_pool(name="slab", bufs=2, space="PSUM"))
        avp = ctx.enter_context(tc.tile_pool(name="avp", bufs=2, space="PSUM"))
        aux = ctx.enter_context(tc.tile_pool(name="aux", bufs=2, space="PSUM"))
        dram = ctx.enter_context(tc.tile_pool(name="dram", bufs=1, space="DRAM"))

        # round-robin assignment of evac-ish work across ACT / DVE
        _exp_ctr = [0]

        def exp_engine():
            c = _exp_ctr[0]
            _exp_ctr[0] += 1
            prev = (c * EXP_DVE_NUM) // EXP_DVE_DEN
            cur = ((c + 1) * EXP_DVE_NUM) // EXP_DVE_DEN
            return "dve" if cur > prev else "act"

        def emit_exp(e_tile, slab_view, eng):
            if eng == "act":
                nc.scalar.activation(e_tile, slab_view, EXPF)
            else:
                nc.vector.tensor_scalar(
                    e_tile.bitcast(i16), slab_view, SCH_C1, SCH_C2, MUL, ADD
                )

        # ---------------- phase 0: x transpose + q projection ------------
        ident = persist.tile([P, P], bf16, tag="ident")
        make_identity(nc, ident[:])
        warm = aux.tile([P, 512], f32, tag="aux", name="warm")
        for _ in range(30):
            nc.tensor.matmul(warm[:, :P], ident[:], ident[:], start=True, stop=True)

        xT = [persist.tile([P, L], bf16, tag=f"xT{ci}", name=f"xT{ci}") for ci in range(4)]
        xv = xd.rearrange("(n p) c -> p n c", p=P)

        def x_quarter(qi):
            st = stage.tile([P, 2, C], f32, tag="stage_x")
            nc.sync.dma_start(st[:], xv[:, 2 * qi : 2 * qi + 2])
            cst = stage.tile([P, 2, C], bf16, tag="stage_xbf")
            nc.scalar.copy(cst[:], st[:])
            for ci in range(4):
                ps = aux.tile([P, 512], bf16, tag="aux", name="xtp")
                for n in range(2):
                    nc.tensor.transpose(
                        ps[:, P * n : P * (n + 1)],
                        cst[:, n, P * ci : P * (ci + 1)],
                        ident[:],
                    )
                nc.vector.tensor_copy(xT[ci][:, 256 * qi : 256 * (qi + 1)], ps[:, :256])

        for qi in range(4):
            x_quarter(qi)

        # xT8: fp8 copy of xT in DoubleRow ktile layout [P, 4, L]
        xT8 = persist.tile([P, 4, L], fp8, tag="xT8")
        for ci in range(4):
            nc.gpsimd.tensor_copy(xT8[:, ci, :], xT[ci][:])

        # Wq -> fp8 [P, 4, C];  bq column tile [P, 4] (prescaled by 0.125)
        wq8 = persist.tile([P, 4, C], fp8, tag="wq8")
        wqv = Wqd.rearrange("(n p) c -> p n c", p=P)
        for i in range(2):
            st = stage.tile([P, 2, C], f32, tag="stage_w")
            nc.sync.dma_start(st[:], wqv[:, 2 * i : 2 * i + 2])
            nc.gpsimd.tensor_copy(wq8[:, 2 * i : 2 * i + 2, :], st[:])
        bq_sb = persist.tile([P, 4], f32, tag="bq")
        nc.sync.dma_start(bq_sb[:], bqd.rearrange("(o p) -> p o", p=P))
        bq_s = persist.tile([P, 4], f32, tag="bqs")
        nc.vector.tensor_scalar(bq_s[:], bq_sb[:], 0.125, None, MUL)

        # q projection (fp8 DR): qT[cq, l] = sum_c Wq[c, cq] xT[c, l]
        # evacuated with scale 0.125 and bias bq*0.125 -> bf16
        qT = [persist.tile([P, L], bf16, tag=f"qT{i}", name=f"qT{i}") for i in range(4)]
        for cq in range(4):
            ps = slab.tile([P, 2, 512], f32, tag="slab", name="qps")
            psv = ps.rearrange("p a n -> p (a n)")
            for lq in range(4):
                for j in range(2):
                    nc.tensor.matmul(
                        psv[:, 256 * lq : 256 * (lq + 1)],
                        wq8[:, 2 * j : 2 * j + 2, P * cq : P * (cq + 1)],
                        xT8[:, 2 * j : 2 * j + 2, 256 * lq : 256 * (lq + 1)],
                        start=(j == 0),
                        stop=(j == 1),
                        perf_mode=DR,
                    )
            nc.scalar.activation(
                qT[cq][:], psv[:], IDF, scale=0.125, bias=bq_s[:, cq : cq + 1]
            )

        # ---------------- phase 0b: post_x / fore_x transposed loads ------
        def cast_to_dram(src, rows, cols):
            # alternate casts between Pool and DVE (both idle-ish at startup)
            n = rows // P
            bf_dram = dram.tile([rows, cols], bf16)
            src_v = src.rearrange("(n p) c -> p n c", p=P)
            dst_v = bf_dram.rearrange("(n p) c -> p n c", p=P)
            step = 4
            for ii, i in enumerate(range(0, n, step)):
                m = min(step, n - i)
                st = stage.tile([P, step, cols], f32, tag="stage_in")
                nc.sync.dma_start(st[:, :m], src_v[:, i : i + m])
                cst = stage.tile([P, step, cols], bf16, tag="stage_bf")
                if ii % 2 == 0:
                    nc.gpsimd.tensor_copy(cst[:, :m], st[:, :m])
                else:
                    nc.vector.tensor_copy(cst[:, :m], st[:, :m])
                nc.sync.dma_start(dst_v[:, i : i + m], cst[:, :m])
            return bf_dram

        def transpose_load_fp8(bf_dram, rows, tagname):
            # -> [P, 2, rows] fp8 (ktile layout for DoubleRow)
            t_bf = tbfp.tile([P, 2, rows], bf16, tag="t_bf", name="t_bf")
            for ci in range(2):
                nc.sync.dma_start_transpose(
                    t_bf[:, ci, :], bf_dram[:, P * ci : P * (ci + 1)]
                )
            t8 = persist.tile([P, 2, rows], fp8, tag=tagname, name=tagname)
            nc.gpsimd.tensor_copy(t8[:, 0, :], t_bf[:, 0, :])
            nc.gpsimd.tensor_copy(t8[:, 1, :], t_bf[:, 1, :])
            return t8

        post_bf = cast_to_dram(postd, S2, CIN)
        postT8 = transpose_load_fp8(post_bf, S2, "postT8")

        def load_wkv8(src, tagname):
            w8 = persist.tile([P, 2, 2 * C], fp8, tag=tagname, name=tagname)
            srcv = src.rearrange("(n p) c -> p n c", p=P)
            for ci in range(2):
                st = stage.tile([P, 2 * C], f32, tag="stage_wkv", name="st")
                nc.sync.dma_start(st[:], srcv[:, ci])
                if ci == 0:
                    nc.vector.tensor_copy(w8[:, ci, :], st[:])
                else:
                    nc.gpsimd.tensor_copy(w8[:, ci, :], st[:])
            return w8

        wkv28 = load_wkv8(Wkv2d, "wkv28")

        # ---------------- kv producers ------------------------------------
        def make_kT(srcT8, w8, S, kname, evac_eng):
            # kT[cq, s] = sum_c Wkv[c, cq] srcT[c, s]   (no bias: cancels)
            tiles = []
            for cq in range(4):
                t = persist.tile([P, S], bf16, tag=f"{kname}{cq}", name=f"{kname}{cq}")
                for off in range(0, S, 1024):
                    w = min(1024, S - off)
                    ps = slab.tile([P, 2, 512], f32, tag="slab", name="kps")
                    psv = ps.rearrange("p a n -> p (a n)")
                    for sb in range(w // 256):
                        nc.tensor.matmul(
                            psv[:, 256 * sb : 256 * (sb + 1)],
                            w8[:, :, P * cq : P * (cq + 1)],
                            srcT8[:, :, off + 256 * sb : off + 256 * (sb + 1)],
                            start=True,
                            stop=True,
                            perf_mode=DR,
                        )
                    dst = t[:, off : off + w]
                    if evac_eng == "alt":
                        evac_eng_i = "act" if (cq + off // 1024) % 2 == 0 else "dve"
                    else:
                        evac_eng_i = evac_eng
                    if evac_eng_i == "act":
                        nc.scalar.copy(dst, psv[:, :w])
                    else:
                        nc.vector.tensor_copy(dst, psv[:, :w])
                tiles.append(t)
            return tiles

        def make_v(srcT8, w8, S, vname):
            # V_aug tiles per si-pair: [P, 2, H, HD+1] bf16 with ones col
            tiles = []
            for u in range(S // 256):
                vt = persist.tile([P, 2, H, HD + 1], bf16, tag=f"{vname}{u}", name=f"{vname}{u}")
                nc.gpsimd.memset(vt[:, :, :, HD : HD + 1], 1.0)
                tiles.append(vt)
            for si in range(S // P):
                ps = aux.tile([P, 512], f32, tag="aux", name="vps")
                for dh in range(2):
                    nc.tensor.matmul(
                        ps[:, 256 * dh : 256 * (dh + 1)],
                        srcT8[:, :, P * si : P * (si + 1)],
                        w8[:, :, C + 256 * dh : C + 256 * (dh + 1)],
                        start=True,
                        stop=True,
                        perf_mode=DR,
                    )
                psv = ps.rearrange("p (h d) -> p h d", h=H)
                dst = tiles[si // 2][:, si % 2, :, 0:HD]
                if si % 2 == 0:
                    nc.vector.tensor_copy(dst, psv[:])
                else:
                    nc.scalar.copy(dst, psv[:])
            return tiles

        kT2 = make_kT(postT8, wkv28, S2, "k2T", "act")
        v2 = make_v(postT8, wkv28, S2, "v2")

        # start the fore_x DRAM round-trip now (DMA + Pool only); the
        # dependent matmuls are emitted after attention(S2) so the PE
        # doesn't stall on these DMAs.
        fore_bf = cast_to_dram(fored, S1, CIN)
        wkv18 = load_wkv8(Wkv1d, "wkv18")

        # ---------------- attention ---------------------------------------
        x1T8 = [persist.tile([P, 2, L], fp8, tag=f"x1T8_{t}", name=f"x1T8_{t}") for t in range(2)]
        x2T8 = [persist.tile([P, 2, L], fp8, tag=f"x2T8_{t}", name=f"x2T8_{t}") for t in range(2)]

        def attention(S, kT, v_sb, xT8_out, after_unit=None):
            # software-pipelined: per si-pair, logits+exp for both heads of
            # the pair are emitted, then the PREVIOUS si-pair's AV matmuls.
            # E tiles are consumed one pipeline stage later, so only ~6 are
            # live and the PE never has to wait for a whole head's exps.
            nsp = S // 256  # number of si-pairs
            for p in range(4):  # head pairs
                for lh in range(2):
                    av_sb = work.tile([P, 4, 2, HD], bf16, tag="av_sb")
                    # PSUM zero-region semantics: a start marks the whole
                    # 2KB window of the tile as pending-zero, so emit start
                    # ONLY on the tile's very first matmul and stop only on
                    # its last; later regions' first writes are handled by
                    # the pending-zero overwrite.
                    avs = [
                        avp.tile([P, 4, HD + 1], f32, tag="av", name=f"av{hh}")
                        for hh in range(2)
                    ]
                    kt = kT[p]
                    qt = qT[p]
                    e_cur = [None, None]

                    def emit_logits_exp(sp):
                        for hh in range(2):
                            po = 64 * hh
                            ps = slab.tile([P, 2, 512], f32, tag="slab", name="lg")
                            for j in range(2):
                                si = 2 * sp + j
                                nc.tensor.matmul(
                                    ps[:, j, :],
                                    kt[po : po + HD, P * si : P * (si + 1)],
                                    qt[po : po + HD, 512 * lh : 512 * (lh + 1)],
                                    start=True,
                                    stop=True,
                                )
                            et = epool.tile([P, 2, 512], bf16, tag="e")
                            emit_exp(
                                et[:], ps.rearrange("p a n -> p (a n)"), exp_engine()
                            )
                            e_cur[hh] = et

                    def emit_av(sp, e_pair):
                        for hh in range(2):
                            for lb in range(4):
                                for j in range(2):
                                    nc.tensor.matmul(
                                        avs[hh][:, lb, :],
                                        e_pair[hh][:, j, P * lb : P * (lb + 1)],
                                        v_sb[sp][:, j, 2 * p + hh, :],
                                        start=(sp == 0 and lb == 0 and j == 0),
                                        stop=(sp == nsp - 1 and lb == 3 and j == 1),
                                        skip_group_check=True,
                                    )

                    if globals()['PIPELINED_AV']:
                        prev = None
                        for sp in range(nsp):
                            emit_logits_exp(sp)
                            if prev is not None:
                                emit_av(prev[0], prev[1])
                            prev = (sp, list(e_cur))
                        emit_av(prev[0], prev[1])
                    else:
                        all_e = []
                        for sp in range(nsp):
                            emit_logits_exp(sp)
                            all_e.append(list(e_cur))
                        for hh in range(2):
                            for lb in range(4):
                                for sp in range(nsp):
                                    for j in range(2):
                                        nc.tensor.matmul(
                                            avs[hh][:, lb, :],
                                            all_e[sp][hh][:, j, P * lb : P * (lb + 1)],
                                            v_sb[sp][:, j, 2 * p + hh, :],
                                            start=(lb == 0 and sp == 0 and j == 0),
                                            stop=(lb == 3 and sp == nsp - 1 and j == 1),
                                            skip_group_check=True,
                                        )

                    # normalize: per-partition reciprocal of ones column
                    for hh in range(2):
                        dnr = work.tile([P, 4], f32, tag="dnr")
                        nc.vector.reciprocal(
                            dnr[:],
                            avs[hh][:, :, HD : HD + 1].rearrange("p a o -> p (a o)"),
                        )
                        for lb in range(4):
                            dst = av_sb[:, lb, hh, :]
                            if (lb + 2 * hh) % 2 == 0:
                                nc.scalar.activation(
                                    dst, avs[hh][:, lb, 0:HD], CPF,
                                    scale=dnr[:, lb : lb + 1],
                                )
                            else:
                                nc.vector.tensor_scalar(
                                    dst, avs[hh][:, lb, 0:HD], dnr[:, lb : lb + 1],
                                    None, MUL,
                                )
                    # transpose back to [c, l] and store as fp8
                    txp = aux.tile([P, 512], bf16, tag="aux", name="txp")
                    for lb in range(4):
                        nc.tensor.transpose(
                            txp[:, P * lb : P * (lb + 1)],
                            av_sb[:, lb, :, :].rearrange("p a d -> p (a d)"),
                            ident[:],
                        )
                    dst = xT8_out[p // 2][:, p % 2, 512 * lh : 512 * (lh + 1)]
                    if p % 2 == 0:
                        nc.scalar.copy(dst, txp[:])
                    else:
                        nc.vector.tensor_copy(dst, txp[:])
                    if after_unit is not None:
                        after_unit(2 * p + lh)

        attention(S2, kT2, v2, x2T8)

        foreT8 = transpose_load_fp8(fore_bf, S1, "foreT8")
        kT1 = make_kT(foreT8, wkv18, S1, "k1T", "alt")
        v1 = make_v(foreT8, wkv18, S1, "v1")

        # ---------------- Wp + bias row -----------------------------------
        wpv = Wpd.rearrange("(n p) c -> p n c", p=P)
        wpx = persist.tile([P, 4, C], bf16, tag="wpx")  # x part, bf16
        for i in range(2):
            st = stage.tile([P, 2, C], f32, tag="stage_w")
            nc.sync.dma_start(st[:], wpv[:, 2 * i : 2 * i + 2])
            nc.gpsimd.tensor_copy(wpx[:, 2 * i : 2 * i + 2, :], st[:])
        wp18 = persist.tile([P, 2, 2, C], fp8, tag="wp18")  # x1 part
        wp28 = persist.tile([P, 2, 2, C], fp8, tag="wp28")  # x2 part
        for t8, base in ((wp18, 4), (wp28, 8)):
            for i in range(2):
                st = stage.tile([P, 2, C], f32, tag="stage_w")
                nc.sync.dma_start(st[:], wpv[:, base + 2 * i : base + 2 * (i + 1)])
                nc.gpsimd.tensor_copy(t8[:, i, :, :], st[:])

        # bias row bpp = bp + bv1 @ Wp[C:2C] + bv2 @ Wp[2C:3C]
        # bv scaled by 16 before fp8 cast (values ~0.02 are subnormal in fp8)
        bv8 = persist.tile([P, 2, 4], fp8, tag="bv8")
        for bi, bd in ((0, bkv1d), (1, bkv2d)):
            bcol = stage.tile([P, 4], f32, tag="stage_bv")
            nc.sync.dma_start(bcol[:], bd[C : 2 * C].rearrange("(o p) -> p o", p=P))
            bscaled = stage.tile([P, 4], f32, tag="stage_bvs")
            nc.vector.tensor_scalar(bscaled[:], bcol[:], 16.0, None, MUL)
            nc.gpsimd.tensor_copy(bv8[:, bi, :], bscaled[:])
        bp_row = persist.tile([1, C], f32, tag="bp_row")
        nc.sync.dma_start(bp_row[:], bpd.rearrange("(o c) -> o c", o=1))
        bias_ps = aux.tile([P, 512], f32, tag="aux", name="biasps")
        for bi, w8 in ((0, wp18), (1, wp28)):
            for k in range(4):
                nc.tensor.matmul(
                    bias_ps[0:1, :],
                    bv8[:, bi, k : k + 1],
                    w8[:, k // 2, k % 2, :],
                    start=(bi == 0 and k == 0),
                    stop=(bi == 1 and k == 3),
                )
        bpp = persist.tile([1, C], bf16, tag="bpp")
        nc.vector.scalar_tensor_tensor(
            bpp[:], bias_ps[0:1, :], 1.0 / 16.0, bp_row[:], MUL, ADD
        )
        onescol = persist.tile([1, P], bf16, tag="onescol")
        nc.vector.memset(onescol[:], 1.0)


        # ---------------- output projection -------------------------------
        # x + x2 + bias partials are emitted at attention-1 unit boundaries
        # (they only need x2T8/xT/bpp); the x1 part runs in the tail.
        acc_sb = [
            persist.tile([P, C], f32, tag=f"acc{li}", name=f"acc{li}")
            for li in range(8)
        ]

        def proj_partial(li):
            ps = aux.tile([P, 512], f32, tag="aux", name="prps")
            nc.tensor.matmul(
                ps[0:P, :], onescol[:, :], bpp[:, :], start=True, stop=False
            )
            for ki in range(4):
                nc.tensor.matmul(
                    ps[:],
                    xT[ki][:, P * li : P * (li + 1)],
                    wpx[:, ki, :],
                    start=False,
                    stop=False,
                )
            for t in range(2):
                for nh in range(2):
                    nc.tensor.matmul(
                        ps[:, 256 * nh : 256 * (nh + 1)],
                        x2T8[t][:, :, P * li : P * (li + 1)],
                        wp28[:, t, :, 256 * nh : 256 * (nh + 1)],
                        start=False,
                        stop=(t == 1 and nh == 1),
                        perf_mode=DR,
                    )
            if li % 2 == 0:
                nc.scalar.copy(acc_sb[li][:], ps[:])
            else:
                nc.vector.tensor_copy(acc_sb[li][:], ps[:])

        attention(
            S1, kT1, v1, x1T8,
            after_unit=lambda u: proj_partial(u) if u < 8 else None,
        )

        for li in range(8):
            ps = aux.tile([P, 512], f32, tag="aux", name="prps2")
            for t in range(2):
                for nh in range(2):
                    nc.tensor.matmul(
                        ps[:, 256 * nh : 256 * (nh + 1)],
                        x1T8[t][:, :, P * li : P * (li + 1)],
                        wp18[:, t, :, 256 * nh : 256 * (nh + 1)],
                        start=(t == 0 and nh == 0),
                        stop=(t == 1 and nh == 1),
                        skip_group_check=True,
                        perf_mode=DR,
                    )
            ot = work.tile([P, C], f32, tag="ot")
            if li % 2 == 0:
                nc.scalar.activation(ot[:], ps[:], IDF, bias=0.0)
                nc.vector.tensor_tensor(ot[:], ot[:], acc_sb[li][:], ADD)
            else:
                nc.vector.tensor_tensor(ot[:], ps[:], acc_sb[li][:], ADD)
            nc.sync.dma_start(outd[P * li : P * (li + 1), :], ot[:])

    if split:
        _split_multi_waits(nc, mybir)
    return nc


def _get_nc():
    if "nc" not in _CACHE:
        _CACHE["nc"] = _build()
    return _CACHE["nc"]


def kernel(**inputs):
    from concourse.bass_utils import run_bass_kernel_spmd

    nc = _get_nc()
    shared = {
        k: np.ascontiguousarray(inputs[k], dtype=np.float32)
        for k in ("Wq", "bq", "Wkv1", "bkv1", "Wkv2", "bkv2", "Wp", "bp")
    }
    in_maps = []
    for b in range(N_CORES):
        m = dict(shared)
        m["x"] = np.ascontiguousarray(inputs["x"][b], dtype=np.float32)
        m["fore_x"] = np.ascontiguousarray(inputs["fore_x"][b], dtype=np.float32)
        m["post_x"] = np.ascontiguousarray(inputs["post_x"][b], dtype=np.float32)
        in_maps.append(m)
    res = run_bass_kernel_spmd(nc, in_maps, core_ids=list(range(N_CORES)))
    out = np.stack([res.results[b]["out"] for b in range(N_CORES)], axis=0)
    return out.astype(np.float32)


# revision 21
# speedup vs baseline: 1.0505x; 1.0300x over previous
"""Trainium2 Bass kernel for nn_CCA2_18786186953483 (dense_transformer).

Reference computation (per batch element b):
  q   = (x @ Wq + bq) * hd^-0.5, split into 8 heads of 64
  x_1 = cross_attn(q, fore_x, Wkv1, bkv1)   # S=2048
  x_2 = cross_attn(q, post_x, Wkv2, bkv2)   # S=512
  out = concat([x, x_1, x_2], -1) @ Wp + bp

Sharding: data-parallel over batch B=8 across the 8 NeuronCores (one batch
element per core); weights replicated.

Kernel strategy (per core):
  * fp8e4m3 DoubleRow matmuls (0.5 cycles/row) for the q / kv projections and
    the x1/x2 parts of the output projection; bf16 for logits, AV, and the
    x part of the output projection.
  * k-bias dropped entirely (adds a per-(head,l) constant to logits, which
    cancels in softmax); v-bias and bp folded into a single bias row added
    via a K=1 ones-row matmul at projection time.
  * Attention AV is computed output-transposed ([l, d] layout) with V
    augmented by a ones column, so softmax denominators land as a per-
    partition column; normalization is a per-partition tensor_scalar
    multiply by the reciprocal, fused into the PSUM->SBUF evacuation.
  * The normalized [l, d] heads are transposed back to [c, l] on the PE
    (identity matmul) for the output projection.
  * Softmax exp is split across two engines: the Activation engine's native
    Exp, and a Schraudolph fast-exp on the Vector engine (fused
    multiply-add to int16, bit-reinterpreted as bf16; ~1.7% rms error).
  * GPSIMD (Pool) does all SBUF-side fp8/bf16 staging casts.
"""

import numpy as np

B, L, C = 8, 1024, 512
H, HD = 8, 64
CIN = 256
S1, S2 = 2048, 512
C3 = 3 * C
P = 128
N_CORES = 8

# Schraudolph fast-exp constants for exp(x) with x the (already 0.125-scaled)
# logits: E = bitcast_bf16(int16(round(x * C1 + C2)))
SCH_C1 = 184.66496280558492  # 128 * log2(e)
SCH_C2 = 16250.5

_CACHE = {}

# exp engine assignment: DVE (Schraudolph) gets EXP_DVE_NUM of every
# EXP_DVE_DEN slabs; the rest go to ACT's native Exp.
EXP_DVE_NUM = 9
EXP_DVE_DEN = 20
PIPELINED_AV = True


def _split_multi_waits(nc, mybir):
    """This container's walrus build supports only ONE sync-wait command per
    instruction ("Too many sync wait commands").  Tile emits instructions
    with several waits; split the extras onto same-engine NOPs placed
    immediately before the instruction (same engine stream => identical
    blocking semantics)."""
    ctr = 0
    for f in nc.m.functions:
        for blk in f.blocks:
            insts = blk.instructions
            if not any(
                i.sync_info is not None
                and i.sync_info.on_wait
                and len(i.sync_info.on_wait) > 1
                for i in insts
            ):
                continue
            new_list = []
            for inst in insts:
                si = inst.sync_info
                waits = list(si.on_wait) if (si is not None and si.on_wait) else []
                if len(waits) > 1:
                    # A wait on the instruction's OWN engine semaphore is
                    # always already satisfied (engines execute and complete
                    # strictly in order), so it is pure dispatch overhead.
                    # Drop it only when doing so avoids emitting split NOPs.
                    own = str(inst.engine).split(".")[-1] + "_"
                    kept = [
                        w
                        for w in waits
                        if not (w.ant_name or "").startswith(own)
                    ]
                    if kept:
                        waits = kept
                if len(waits) == 1:
                    inst.sync_info = mybir.SyncInfo(
                        on_wait=waits, on_update=list(si.on_update or [])
                    )
                if len(waits) > 1:
                    for w in waits[:-1]:
                        ctr += 1
                        new_list.append(
                            mybir.InstNoOp(
                                name=f"I-waitsplit-{ctr}",
                                engine=inst.engine,
                                bass_nofuse=True,
                                sync_info=mybir.SyncInfo(on_wait=[w], on_update=[]),
                            )
                        )
                    inst.sync_info = mybir.SyncInfo(
                        on_wait=[waits[-1]], on_update=list(si.on_update or [])
                    )
                new_list.append(inst)
            insts[:] = new_list
    return ctr


def _build(split=True):
    import contextlib

    import concourse.bass as bass
    import concourse.tile as tile
    from concourse import mybir
    from concourse.masks import make_identity

    f32 = mybir.dt.float32
    bf16 = mybir.dt.bfloat16
    fp8 = mybir.dt.float8e4
    i16 = mybir.dt.int16
    EXPF = mybir.ActivationFunctionType.Exp
    IDF = mybir.ActivationFunctionType.Identity
    CPF = mybir.ActivationFunctionType.Copy
    DR = mybir.MatmulPerfMode.DoubleRow
    MUL = mybir.AluOpType.mult
    ADD = mybir.AluOpType.add

    nc = bass.Bass("TRN2")

    xd = nc.dram_tensor("x", [L, C], f32, kind="ExternalInput")
    fored = nc.dram_tensor("fore_x", [S1, CIN], f32, kind="ExternalInput")
    postd = nc.dram_tensor("post_x", [S2, CIN], f32, kind="ExternalInput")
    Wqd = nc.dram_tensor("Wq", [C, C], f32, kind="ExternalInput")
    bqd = nc.dram_tensor("bq", [C], f32, kind="ExternalInput")
    Wkv1d = nc.dram_tensor("Wkv1", [CIN, 2 * C], f32, kind="ExternalInput")
    bkv1d = nc.dram_tensor("bkv1", [2 * C], f32, kind="ExternalInput")
    Wkv2d = nc.dram_tensor("Wkv2", [CIN, 2 * C], f32, kind="ExternalInput")
    bkv2d = nc.dram_tensor("bkv2", [2 * C], f32, kind="ExternalInput")
    Wpd = nc.dram_tensor("Wp", [C3, C], f32, kind="ExternalInput")
    bpd = nc.dram_tensor("bp", [C], f32, kind="ExternalInput")
    outd = nc.dram_tensor("out", [L, C], f32, kind="ExternalOutput")

    with tile.TileContext(nc) as tc, contextlib.ExitStack() as ctx:
        stage = ctx.enter_context(tc.tile_pool(name="stage", bufs=3))
        tbfp = ctx.enter_context(tc.tile_pool(name="tbfp", bufs=1))
        epool = ctx.enter_context(tc.tile_pool(name="epool", bufs=11))
        work = ctx.enter_context(tc.tile_pool(name="work", bufs=3))
        persist = ctx.enter_context(tc.tile_pool(name="persist", bufs=1))
        slab = ctx.enter_context(tc.tile_pool(name="slab", bufs=6, space="PSUM"))
        avp = ctx.enter_context(tc.tile_pool(name="avp", bufs=2, space="PSUM"))
        dram = ctx.enter_context(tc.tile_pool(name="dram", bufs=1, space="DRAM"))

        # round-robin assignment of evac-ish work across ACT / DVE
        _exp_ctr = [0]

        def exp_engine():
            c = _exp_ctr[0]
            _exp_ctr[0] += 1
            prev = (c * EXP_DVE_NUM) // EXP_DVE_DEN
            cur = ((c + 1) * EXP_DVE_NUM) // EXP_DVE_DEN
            return "dve" if cur > prev else "act"

        def emit_exp(e_tile, slab_view, eng):
            if eng == "act":
                nc.scalar.activation(e_tile, slab_view, EXPF)
            else:
                nc.vector.tensor_scalar(
                    e_tile.bitcast(i16), slab_view, SCH_C1, SCH_C2, MUL, ADD
                )

        # ---------------- phase 0: x transpose + q projection ------------
        ident = persist.tile([P, P], bf16, tag="ident")
        make_identity(nc, ident[:])
        warm = slab.tile([P, 512], f32, tag="slab", name="warm")
        for _ in range(30):
            nc.tensor.matmul(warm[:, :P], ident[:], ident[:], start=True, stop=True)

        xT = [persist.tile([P, L], bf16, tag=f"xT{ci}", name=f"xT{ci}") for ci in range(4)]
        xv = xd.rearrange("(n p) c -> p n c", p=P)

        def x_quarter(qi):
            st = stage.tile([P, 2, C], f32, tag="stage_x")
            nc.sync.dma_start(st[:], xv[:, 2 * qi : 2 * qi + 2])
            cst = stage.tile([P, 2, C], bf16, tag="stage_xbf")
            nc.scalar.copy(cst[:], st[:])
            for ci in range(4):
                ps = slab.tile([P, 512], bf16, tag="slab", name="xtp")
                for n in range(2):
                    nc.tensor.transpose(
                        ps[:, P * n : P * (n + 1)],
                        cst[:, n, P * ci : P * (ci + 1)],
                        ident[:],
                    )
                nc.vector.tensor_copy(xT[ci][:, 256 * qi : 256 * (qi + 1)], ps[:, :256])

        for qi in range(4):
            x_quarter(qi)

        # xT8: fp8 copy of xT in DoubleRow ktile layout [P, 4, L]
        xT8 = persist.tile([P, 4, L], fp8, tag="xT8")
        for ci in range(4):
            nc.gpsimd.tensor_copy(xT8[:, ci, :], xT[ci][:])

        # Wq -> fp8 [P, 4, C];  bq column tile [P, 4] (prescaled by 0.125)
        wq8 = persist.tile([P, 4, C], fp8, tag="wq8")
        wqv = Wqd.rearrange("(n p) c -> p n c", p=P)
        for i in range(2):
            st = stage.tile([P, 2, C], f32, tag="stage_w")
            nc.sync.dma_start(st[:], wqv[:, 2 * i : 2 * i + 2])
            nc.gpsimd.tensor_copy(wq8[:, 2 * i : 2 * i + 2, :], st[:])
        bq_sb = persist.tile([P, 4], f32, tag="bq")
        nc.sync.dma_start(bq_sb[:], bqd.rearrange("(o p) -> p o", p=P))
        bq_s = persist.tile([P, 4], f32, tag="bqs")
        nc.vector.tensor_scalar(bq_s[:], bq_sb[:], 0.125, None, MUL)

        # q projection (fp8 DR): qT[cq, l] = sum_c Wq[c, cq] xT[c, l]
        # evacuated with scale 0.125 and bias bq*0.125 -> bf16
        qT = [persist.tile([P, L], bf16, tag=f"qT{i}", name=f"qT{i}") for i in range(4)]
        for cq in range(4):
            for lh in range(2):
                ps = slab.tile([P, 512], f32, tag="slab", name="qps")
                for lq in range(2):
                    for j in range(2):
                        o = 512 * lh + 256 * lq
                        nc.tensor.matmul(
                            ps[:, 256 * lq : 256 * (lq + 1)],
                            wq8[:, 2 * j : 2 * j + 2, P * cq : P * (cq + 1)],
                            xT8[:, 2 * j : 2 * j + 2, o : o + 256],
                            start=(j == 0),
                            stop=(j == 1),
                            perf_mode=DR,
                        )
                nc.scalar.activation(
                    qT[cq][:, 512 * lh : 512 * (lh + 1)], ps[:], IDF,
                    scale=0.125, bias=bq_s[:, cq : cq + 1],
                )

        # ---------------- phase 0b: post_x / fore_x transposed loads ------
        def cast_to_dram(src, rows, cols):
            # alternate casts between Pool and DVE (both idle-ish at startup)
            n = rows // P
            bf_dram = dram.tile([rows, cols], bf16)
            src_v = src.rearrange("(n p) c -> p n c", p=P)
            dst_v = bf_dram.rearrange("(n p) c -> p n c", p=P)
            step = 4
            for ii, i in enumerate(range(0, n, step)):
                m = min(step, n - i)
                st = stage.tile([P, step, cols], f32, tag="stage_in")
                nc.sync.dma_start(st[:, :m], src_v[:, i : i + m])
                cst = stage.tile([P, step, cols], bf16, tag="stage_bf")
                if ii % 2 == 0:
                    nc.gpsimd.tensor_copy(cst[:, :m], st[:, :m])
                else:
                    nc.vector.tensor_copy(cst[:, :m], st[:, :m])
                nc.sync.dma_start(dst_v[:, i : i + m], cst[:, :m])
            return bf_dram

        def transpose_load_fp8(bf_dram, rows, tagname):
            # -> [P, 2, rows] fp8 (ktile layout for DoubleRow)
            t_bf = tbfp.tile([P, 2, rows], bf16, tag="t_bf", name="t_bf")
            for ci in range(2):
                nc.sync.dma_start_transpose(
                    t_bf[:, ci, :], bf_dram[:, P * ci : P * (ci + 1)]
                )
            t8 = persist.tile([P, 2, rows], fp8, tag=tagname, name=tagname)
            nc.gpsimd.tensor_copy(t8[:, 0, :], t_bf[:, 0, :])
            nc.gpsimd.tensor_copy(t8[:, 1, :], t_bf[:, 1, :])
            return t8

        post_bf = cast_to_dram(postd, S2, CIN)
        postT8 = transpose_load_fp8(post_bf, S2, "postT8")

        def load_wkv8(src, tagname):
            w8 = persist.tile([P, 2, 2 * C], fp8, tag=tagname, name=tagname)
            srcv = src.rearrange("(n p) c -> p n c", p=P)
            for ci in range(2):
                st = stage.tile([P, 2 * C], f32, tag="stage_wkv", name="st")
                nc.sync.dma_start(st[:], srcv[:, ci])
                if ci == 0:
                    nc.vector.tensor_copy(w8[:, ci, :], st[:])
                else:
                    nc.gpsimd.tensor_copy(w8[:, ci, :], st[:])
            return w8

        wkv28 = load_wkv8(Wkv2d, "wkv28")

        # ---------------- kv producers ------------------------------------
        def make_kT(srcT8, w8, S, kname, evac_eng):
            # kT[cq, s] = sum_c Wkv[c, cq] srcT[c, s]   (no bias: cancels)
            tiles = []
            for cq in range(4):
                t = persist.tile([P, S], bf16, tag=f"{kname}{cq}", name=f"{kname}{cq}")
                for off in range(0, S, 512):
                    w = min(512, S - off)
                    ps = slab.tile([P, 512], f32, tag="slab", name="kps")
                    for sb in range(w // 256):
                        nc.tensor.matmul(
                            ps[:, 256 * sb : 256 * (sb + 1)],
                            w8[:, :, P * cq : P * (cq + 1)],
                            srcT8[:, :, off + 256 * sb : off + 256 * (sb + 1)],
                            start=True,
                            stop=True,
                            perf_mode=DR,
                        )
                    dst = t[:, off : off + w]
                    if evac_eng == "alt":
                        evac_eng_i = "act" if (cq + off // 512) % 2 == 0 else "dve"
                    else:
                        evac_eng_i = evac_eng
                    if evac_eng_i == "act":
                        nc.scalar.copy(dst, ps[:, :w])
                    else:
                        nc.vector.tensor_copy(dst, ps[:, :w])
                tiles.append(t)
            return tiles

        def make_v(srcT8, w8, S, vname):
            # V_aug tiles per si-pair: [P, 2, H, HD+1] bf16 with ones col
            tiles = []
            for u in range(S // 256):
                vt = persist.tile([P, 2, H, HD + 1], bf16, tag=f"{vname}{u}", name=f"{vname}{u}")
                nc.gpsimd.memset(vt[:, :, :, HD : HD + 1], 1.0)
                tiles.append(vt)
            for si in range(S // P):
                ps = slab.tile([P, 512], f32, tag="slab", name="vps")
                for dh in range(2):
                    nc.tensor.matmul(
                        ps[:, 256 * dh : 256 * (dh + 1)],
                        srcT8[:, :, P * si : P * (si + 1)],
                        w8[:, :, C + 256 * dh : C + 256 * (dh + 1)],
                        start=True,
                        stop=True,
                        perf_mode=DR,
                    )
                psv = ps.rearrange("p (h d) -> p h d", h=H)
                dst = tiles[si // 2][:, si % 2, :, 0:HD]
                if si % 2 == 0:
                    nc.vector.tensor_copy(dst, psv[:])
                else:
                    nc.scalar.copy(dst, psv[:])
            return tiles

        kT2 = make_kT(postT8, wkv28, S2, "k2T", "act")
        v2 = make_v(postT8, wkv28, S2, "v2")

        # start the fore_x DRAM round-trip now (DMA + Pool only); the
        # dependent matmuls are emitted after attention(S2) so the PE
        # doesn't stall on these DMAs.
        fore_bf = cast_to_dram(fored, S1, CIN)
        wkv18 = load_wkv8(Wkv1d, "wkv18")

        # ---------------- attention ---------------------------------------
        x1T8 = [persist.tile([P, 2, L], fp8, tag=f"x1T8_{t}", name=f"x1T8_{t}") for t in range(2)]
        x2T8 = [persist.tile([P, 2, L], fp8, tag=f"x2T8_{t}", name=f"x2T8_{t}") for t in range(2)]

        def attention(S, kT, v_sb, xT8_out, after_unit=None):
            # software-pipelined: per si-pair, logits+exp for both heads of
            # the pair are emitted, then the PREVIOUS si-pair's AV matmuls.
            # E tiles are consumed one pipeline stage later, so only ~6 are
            # live and the PE never has to wait for a whole head's exps.
            nsp = S // 256  # number of si-pairs
            for p in range(4):  # head pairs
                for lh in range(2):
                    av_sb = work.tile([P, 4, 2, HD], bf16, tag="av_sb")
                    # PSUM zero-region semantics: a start marks the whole
                    # 2KB window of the tile as pending-zero, so emit start
                    # ONLY on the tile's very first matmul and stop only on
                    # its last; later regions' first writes are handled by
                    # the pending-zero overwrite.
                    avs = [
                        avp.tile([P, 4, HD + 1], f32, tag="av", name=f"av{hh}")
                        for hh in range(2)
                    ]
                    kt = kT[p]
                    qt = qT[p]
                    e_cur = [None, None]

                    def emit_logits_exp(sp):
                        for hh in range(2):
                            po = 64 * hh
                            et = epool.tile([P, 2, 512], bf16, tag="e")
                            for j in range(2):
                                si = 2 * sp + j
                                ps = slab.tile([P, 512], f32, tag="slab", name="lg")
                                nc.tensor.matmul(
                                    ps[:],
                                    kt[po : po + HD, P * si : P * (si + 1)],
                                    qt[po : po + HD, 512 * lh : 512 * (lh + 1)],
                                    start=True,
                                    stop=True,
                                )
                                emit_exp(et[:, j, :], ps[:], exp_engine())
                            e_cur[hh] = et

                    def emit_av(sp, e_pair):
                        for hh in range(2):
                            for lb in range(4):
                                for j in range(2):
                                    nc.tensor.matmul(
                                        avs[hh][:, lb, :],
                                        e_pair[hh][:, j, P * lb : P * (lb + 1)],
                                        v_sb[sp][:, j, 2 * p + hh, :],
                                        start=(sp == 0 and lb == 0 and j == 0),
                                        stop=(sp == nsp - 1 and lb == 3 and j == 1),
                                        skip_group_check=True,
                                    )

                    if globals()['PIPELINED_AV']:
                        prev = None
                        for sp in range(nsp):
                            emit_logits_exp(sp)
                            if prev is not None:
                                emit_av(prev[0], prev[1])
                            prev = (sp, list(e_cur))
                        emit_av(prev[0], prev[1])
                    else:
                        all_e = []
                        for sp in range(nsp):
                            emit_logits_exp(sp)
                            all_e.append(list(e_cur))
                        for hh in range(2):
                            for lb in range(4):
                                for sp in range(nsp):
                                    for j in range(2):
                                        nc.tensor.matmul(
                                            avs[hh][:, lb, :],
                                            all_e[sp][hh][:, j, P * lb : P * (lb + 1)],
                                            v_sb[sp][:, j, 2 * p + hh, :],
                                            start=(lb == 0 and sp == 0 and j == 0),
                                            stop=(lb == 3 and sp == nsp - 1 and j == 1),
                                            skip_group_check=True,
                                        )

                    # normalize: per-partition reciprocal of ones column
                    for hh in range(2):
                        dnr = work.tile([P, 4], f32, tag="dnr")
                        nc.vector.reciprocal(
                            dnr[:],
                            avs[hh][:, :, HD : HD + 1].rearrange("p a o -> p (a o)"),
                        )
                        for lb in range(4):
                            dst = av_sb[:, lb, hh, :]
                            if (lb + 2 * hh) % 2 == 0:
                                nc.scalar.activation(
                                    dst, avs[hh][:, lb, 0:HD], CPF,
                                    scale=dnr[:, lb : lb + 1],
                                )
                            else:
                                nc.vector.tensor_scalar(
                                    dst, avs[hh][:, lb, 0:HD], dnr[:, lb : lb + 1],
                                    None, MUL,
                                )
                    # transpose back to [c, l] and store as fp8
                    txp = slab.tile([P, 512], bf16, tag="slab", name="txp")
                    for lb in range(4):
                        nc.tensor.transpose(
                            txp[:, P * lb : P * (lb + 1)],
                            av_sb[:, lb, :, :].rearrange("p a d -> p (a d)"),
                            ident[:],
                        )
                    dst = xT8_out[p // 2][:, p % 2, 512 * lh : 512 * (lh + 1)]
                    if p % 2 == 0:
                        nc.scalar.copy(dst, txp[:])
                    else:
                        nc.vector.tensor_copy(dst, txp[:])
                    if after_unit is not None:
                        after_unit(2 * p + lh)

        attention(S2, kT2, v2, x2T8)

        foreT8 = transpose_load_fp8(fore_bf, S1, "foreT8")
        kT1 = make_kT(foreT8, wkv18, S1, "k1T", "alt")
        v1 = make_v(foreT8, wkv18, S1, "v1")

        # ---------------- Wp + bias row -----------------------------------
        wpv = Wpd.rearrange("(n p) c -> p n c", p=P)
        wpx = persist.tile([P, 4, C], bf16, tag="wpx")  # x part, bf16
        for i in range(2):
            st = stage.tile([P, 2, C], f32, tag="stage_w")
            nc.sync.dma_start(st[:], wpv[:, 2 * i : 2 * i + 2])
            nc.gpsimd.tensor_copy(wpx[:, 2 * i : 2 * i + 2, :], st[:])
        wp18 = persist.tile([P, 2, 2, C], fp8, tag="wp18")  # x1 part
        wp28 = persist.tile([P, 2, 2, C], fp8, tag="wp28")  # x2 part
        for t8, base in ((wp18, 4), (wp28, 8)):
            for i in range(2):
                st = stage.tile([P, 2, C], f32, tag="stage_w")
                nc.sync.dma_start(st[:], wpv[:, base + 2 * i : base + 2 * (i + 1)])
                nc.gpsimd.tensor_copy(t8[:, i, :, :], st[:])

        # bias row bpp = bp + bv1 @ Wp[C:2C] + bv2 @ Wp[2C:3C]
        # bv scaled by 16 before fp8 cast (values ~0.02 are subnormal in fp8)
        bv8 = persist.tile([P, 2, 4], fp8, tag="bv8")
        for bi, bd in ((0, bkv1d), (1, bkv2d)):
            bcol = stage.tile([P, 4], f32, tag="stage_bv")
            nc.sync.dma_start(bcol[:], bd[C : 2 * C].rearrange("(o p) -> p o", p=P))
            bscaled = stage.tile([P, 4], f32, tag="stage_bvs")
            nc.vector.tensor_scalar(bscaled[:], bcol[:], 16.0, None, MUL)
            nc.gpsimd.tensor_copy(bv8[:, bi, :], bscaled[:])
        bp_row = persist.tile([1, C], f32, tag="bp_row")
        nc.sync.dma_start(bp_row[:], bpd.rearrange("(o c) -> o c", o=1))
        bias_ps = slab.tile([P, 512], f32, tag="slab", name="biasps")
        for bi, w8 in ((0, wp18), (1, wp28)):
            for k in range(4):
                nc.tensor.matmul(
                    bias_ps[0:1, :],
                    bv8[:, bi, k : k + 1],
                    w8[:, k // 2, k % 2, :],
                    start=(bi == 0 and k == 0),
                    stop=(bi == 1 and k == 3),
                )
        bpp = persist.tile([1, C], bf16, tag="bpp")
        nc.vector.scalar_tensor_tensor(
            bpp[:], bias_ps[0:1, :], 1.0 / 16.0, bp_row[:], MUL, ADD
        )
        onescol = persist.tile([1, P], bf16, tag="onescol")
        nc.vector.memset(onescol[:], 1.0)


        # ---------------- output projection -------------------------------
        # x + x2 + bias partials are emitted at attention-1 unit boundaries
        # (they only need x2T8/xT/bpp); the x1 part runs in the tail.
        acc_sb = [
            persist.tile([P, C], f32, tag=f"acc{li}", name=f"acc{li}")
            for li in range(8)
        ]

        def proj_partial(li):
            ps = slab.tile([P, 512], f32, tag="slab", name="prps")
            nc.tensor.matmul(
                ps[0:P, :], onescol[:, :], bpp[:, :], start=True, stop=False
            )
            for ki in range(4):
                nc.tensor.matmul(
                    ps[:],
                    xT[ki][:, P * li : P * (li + 1)],
                    wpx[:, ki, :],
                    start=False,
                    stop=False,
                )
            for t in range(2):
                for nh in range(2):
                    nc.tensor.matmul(
                        ps[:, 256 * nh : 256 * (nh + 1)],
                        x2T8[t][:, :, P * li : P * (li + 1)],
                        wp28[:, t, :, 256 * nh : 256 * (nh + 1)],
                        start=False,
                        stop=(t == 1 and nh == 1),
                        perf_mode=DR,
                    )
            if li % 2 == 0:
                nc.scalar.copy(acc_sb[li][:], ps[:])
            else:
                nc.vector.tensor_copy(acc_sb[li][:], ps[:])

        attention(
            S1, kT1, v1, x1T8,
            after_unit=lambda u: proj_partial(u) if u < 8 else None,
        )

        for li in range(8):
            ps = slab.tile([P, 512], f32, tag="slab", name="prps2")
            for t in range(2):
                for nh in range(2):
                    nc.tensor.matmul(
                        ps[:, 256 * nh : 256 * (nh + 1)],
                        x1T8[t][:, :, P * li : P * (li + 1)],
                        wp18[:, t, :, 256 * nh : 256 * (nh + 1)],
                        start=(t == 0 and nh == 0),
                        stop=(t == 1 and nh == 1),
                        skip_group_check=True,
                        perf_mode=DR,
                    )
            ot = work.tile([P, C], f32, tag="ot")
            if li % 2 == 0:
                nc.scalar.activation(ot[:], ps[:], IDF, bias=0.0)
                nc.vector.tensor_tensor(ot[:], ot[:], acc_sb[li][:], ADD)
            else:
                nc.vector.tensor_tensor(ot[:], ps[:], acc_sb[li][:], ADD)
            nc.sync.dma_start(outd[P * li : P * (li + 1), :], ot[:])

    if split:
        _split_multi_waits(nc, mybir)
    return nc


def _get_nc():
    if "nc" not in _CACHE:
        _CACHE["nc"] = _build()
    return _CACHE["nc"]


def kernel(**inputs):
    from concourse.bass_utils import run_bass_kernel_spmd

    nc = _get_nc()
    shared = {
        k: np.ascontiguousarray(inputs[k], dtype=np.float32)
        for k in ("Wq", "bq", "Wkv1", "bkv1", "Wkv2", "bkv2", "Wp", "bp")
    }
    in_maps = []
    for b in range(N_CORES):
        m = dict(shared)
        m["x"] = np.ascontiguousarray(inputs["x"][b], dtype=np.float32)
        m["fore_x"] = np.ascontiguousarray(inputs["fore_x"][b], dtype=np.float32)
        m["post_x"] = np.ascontiguousarray(inputs["post_x"][b], dtype=np.float32)
        in_maps.append(m)
    res = run_bass_kernel_spmd(nc, in_maps, core_ids=list(range(N_CORES)))
    out = np.stack([res.results[b]["out"] for b in range(N_CORES)], axis=0)
    return out.astype(np.float32)
